# revision 50
# baseline (speedup 1.0000x reference)
"""
Trainium2 Bass kernel for DynamicGraphAttention
(softmax(Hn Wq^T (Hn Wk^T)^T / sqrt(D) + eta*logit(clip(A)) masked)).

Shapes (hardcoded):
  Hn     [16, 2048, 256] f32
  A_stat [2048, 2048]    f32
  M_mask [2048, 2048]    int32
  Wq, Wk [256, 256]      f32
  out    [16, 2048, 2048] f32

Sharding across 8 NeuronCores: 4 batch-groups x 2 seq(query)-groups.
Core c handles batches of group bg = c // 2 (4 batches) and query rows
[qg*1024:(qg+1)*1024] (qg = c % 2). The program is identical on all
cores (SPMD): for qg=1 cores the host swaps the two key-column halves
of hnt and w so the core's own query block is always columns [0:1024],
and the assemble step swaps the output's key axis back.

Hybrid bias application, chosen per qt slot to balance PE/ACT/DVE:
  mult slots: softmax(S+b) == exp(S)*w / sum(exp(S)*w) with
      w = mask * a/(1-a)  (fp16, host-computed; masked -> exactly 0).
      DVE AFFINE_MUL_REDUCE fuses pw = exp(S)*w with the rowsum.
  ADD_QT slots: btab = ln(w) is added into PSUM via an identity
      matmul on the PE (which has spare throughput there), and the
      ACT exp accumulates the rowsum for free.

G = (Wq^T Wk)/sqrt(D) is folded host-side (weight-only preprocessing,
fp16 [128,2,256]); Hn ships pre-transposed fp16 and the query block is
a column-slice view of it (no separate hqt load).

Device algorithm (per core):
  VT   = G^T HqT  per batch, fp16          [256,1024]        (PE)
  S    = VT.T @ HnT  (fp16 matmuls) PSUM f32 (+btab on ADD)  (PE)
  e    = exp(S) -> SBUF fp16 (+rowsum accum on ADD slots)    (ACT)
  pw   = e * w[qt], rs = rowsum(pw)   (AFFINE_MUL_REDUCE)    (DVE)
  out  = pw * (1/rs)  bf16 -> DRAM                           (DVE)

Emission order = per-engine execution order; VT builds (PSUM vp pair
adjacent to keep ring parity; casts on ACT so the DVE FIFO can't delay
them) and the ring-split input DMAs are interleaved into the sweep.
Output is bf16 on device (fp16 lacks exponent range for the smallest
softmax probabilities), upcast to f32 on host.
"""

import math

import numpy as np

import concourse.bass as bass
import concourse.bacc as bacc
import concourse.tile as tile
from concourse import mybir
from concourse import bass_utils

F32 = mybir.dt.float32
FP16 = mybir.dt.float16
BF16 = mybir.dt.bfloat16

B_FULL = 16
N = 2048
D = 256
NBG = 4   # batch groups
NQG = 2   # seq (query-row) groups
NB = B_FULL // NBG        # batches per core = 4
NQ = N // NQG             # query rows per core = 1024
NQT = NQ // 128           # q tiles per core = 8
EPS = 1e-3
SCALE = 1.0 / math.sqrt(float(D))  # 1/16

# qt slots using the additive ln(w) bias via PE identity-matmul (the rest
# multiply by w on the DVE); qt slots whose normalize runs on ACT.
# Chosen to balance PE / ACT / DVE busy time.
ADD_QT = (1, 4, 7)
ACT_NORM_QT = ()


def _is_add(b, qt):
    return qt in ADD_QT

_CACHE = {}


def _patch_act_tables():
    # Prefer the activation-table set that holds Exp (+Copy) so the
    # scalar engine never reloads tables mid-kernel.
    from concourse import hw_specs as _hw
    if getattr(_hw, "_combined_first", False):
        return
    _orig = _hw.get_activation_tables

    def _patched(module_arch):
        tabs = _orig(module_arch)
        pref = "natural_log_exp_and_others"
        if pref in tabs:
            both = {mybir.ActivationFunctionType.Ln,
                    mybir.ActivationFunctionType.Exp,
                    mybir.ActivationFunctionType.Copy}
            tabs = {
                k: (v if k == pref else (v - both))
                for k, v in tabs.items()
            }
        return tabs

    _hw.get_activation_tables = _patched
    import concourse.bacc as _bacc_mod
    _bacc_mod.get_activation_tables = _patched
    _hw._combined_first = True


def _build():
    _patch_act_tables()
    nc = bacc.Bacc("TRN2", debug=False, enable_asserts=False)

    hnt_d = nc.dram_tensor("hnt", [NB, D, N], FP16, kind="ExternalInput").ap()
    # per-qt-slot bias sheet: ln(w) for ADD_QT slots, w for the others
    w_d = nc.dram_tensor("w", [NQT, 128, N], FP16, kind="ExternalInput").ap()
    g_d = nc.dram_tensor("g", [128, 2, D], FP16, kind="ExternalInput").ap()
    idb_d = nc.dram_tensor("idb", [128, 128], FP16, kind="ExternalInput").ap()
    o_d = nc.dram_tensor("o", [NB, NQ, N], BF16, kind="ExternalOutput").ap()

    with tile.TileContext(nc) as tc:
        with (
            tc.tile_pool(name="consts", bufs=1) as consts,
            tc.tile_pool(name="wp", bufs=NQT) as wp,
            tc.tile_pool(name="hntp", bufs=12) as hntp,
            tc.tile_pool(name="vtp", bufs=4) as vtp,
            tc.tile_pool(name="pp", bufs=3) as pp,
            tc.tile_pool(name="pwp", bufs=3) as pwp,
            tc.tile_pool(name="op", bufs=3) as op_pool,
            tc.tile_pool(name="rsp", bufs=8) as rsp,
            tc.tile_pool(name="ps_s", bufs=2, space="PSUM") as ps_s,
        ):
            # ---- PE p-state warmup, first thing in every engine stream:
            # the full-speed p-state window opens only after sustained PE
            # activity, so the warmup matmuls must not wait on a DVE memset.
            # They read idb garbage-before-DMA on purpose (weights values
            # are irrelevant; the PSUM slot is reset by later start=True).
            wrm = consts.tile([128, 128], FP16, tag="wrm")
            wps = ps_s.tile([128, N], F32, tag="s", name="warmps")
            for _ in range(18):
                nc.tensor.matmul(
                    wps[:, :128], lhsT=wrm, rhs=wrm, start=True, stop=True,
                    skip_group_check=True,
                )

            # ---- tiny const + ACT table preload (Exp) ----
            tinyc = consts.tile([128, 1], F32, tag="tiny")
            nc.vector.memset(tinyc, 1e-30)
            nc.vector.memset(wrm, 0.5)
            warm = consts.tile([128, 1], F32, tag="warm")
            nc.scalar.activation(
                out=warm, in_=tinyc,
                func=mybir.ActivationFunctionType.Exp, bias=0.0, scale=1.0,
            )

            # ---- constants ----
            g_sb = consts.tile([128, 2, D], FP16, tag="g")
            nc.sync.dma_start(out=g_sb, in_=g_d)
            idb = consts.tile([128, 128], FP16, tag="idb")
            nc.sync.dma_start(out=idb, in_=idb_d)

            # DMA rings: sync + act are HWDGE, gps is SWDGE (also carries
            # the output stores). Ramp loads are split across all three.
            rings = {"sync": nc.sync, "act": nc.scalar, "gps": nc.gpsimd}

            # w tiles
            w_tiles = {}

            def emit_w(t, ring="gps"):
                w_t = wp.tile([128, N], FP16, tag="w", name=f"w{t}")
                rings[ring].dma_start(out=w_t, in_=w_d[t])
                w_tiles[t] = w_t

            # hnt[b] as 4 subtiles [i][h]: i = d'-row block, h = col half.
            # Half 0 is always the core's own query block (host-permuted),
            # loaded first so VT can start early.
            hnts = {}

            def emit_hnt(b, half=None, ring="sync"):
                halves = [0, 1] if half is None else [half]
                if b not in hnts:
                    hnts[b] = [[None, None], [None, None]]
                for h in halves:
                    for i in range(2):
                        t = hntp.tile([128, NQ], FP16, tag="hnt",
                                      name=f"hnt{b}_{i}{h}")
                        rings[ring].dma_start(
                            out=t,
                            in_=hnt_d[b, i * 128:(i + 1) * 128,
                                      h * NQ:(h + 1) * NQ],
                        )
                        hnts[b][i][h] = t
                return hnts[b]

            # preamble loads spread over the three rings so batch 0+1 and
            # the first bias sheets land as fast as possible; b0h0's two
            # subtiles ride different rings so they land in parallel
            t00 = hntp.tile([128, NQ], FP16, tag="hnt", name="hnt0_00")
            nc.gpsimd.dma_start(out=t00, in_=hnt_d[0, 0:128, 0:NQ])
            t01 = hntp.tile([128, NQ], FP16, tag="hnt", name="hnt0_10")
            nc.scalar.dma_start(out=t01, in_=hnt_d[0, 128:256, 0:NQ])
            hnts[0] = [[t00, None], [t01, None]]
            emit_hnt(0, half=1, ring="act")
            emit_w(1, "gps")
            emit_w(0, "gps")

            emit_hnt(1, half=0, ring="sync")
            emit_hnt(1, half=1, ring="sync")
            emit_w(2, "gps")
            emit_w(3, "gps")
            emit_w(6, "act")
            emit_w(7, "act")

            # ---- VT builder: VT[d',q] = sum_d G[d,d'] HqT[d,q] ----
            # vp PSUM written in 512-col chunks (bank-sized matmuls);
            # copy engine selectable for load balancing.
            def emit_vt(b, engs=("dve", "dve")):
                vt = []
                hq = [hnts[b][i][0] for i in range(2)]
                for j in range(2):
                    vt_j = vtp.tile([128, NQ], FP16, tag="vt", name=f"vt{b}_{j}")
                    vp = ps_s.tile(
                        [128, N], F32, tag="s", name=f"vp{b}{j}"
                    )[:, :NQ]
                    for i in range(2):
                        for c in range(2):
                            csl = slice(c * 512, (c + 1) * 512)
                            nc.tensor.matmul(
                                vp[:, csl],
                                lhsT=g_sb[:, i, j * 128:(j + 1) * 128],
                                rhs=hq[i][:, csl],
                                start=(i == 0),
                                stop=(i == 1),
                            )
                    if engs[j] == "act":
                        nc.scalar.copy(out=vt_j, in_=vp)
                    else:
                        nc.vector.tensor_copy(out=vt_j, in_=vp)
                    vt.append(vt_j)
                return vt

            vts = {}

            def emit_qtile_s(b, qt):
                qsl = slice(qt * 128, (qt + 1) * 128)
                vt, hnt = vts[b], hnts[b]
                add_bias = _is_add(b, qt)
                s_ps = ps_s.tile([128, N], F32, tag="s", name=f"s{b}{qt}")
                for j in range(2):
                    for c in range(4):
                        csl = slice(c * 512, (c + 1) * 512)
                        rhs = hnt[j][c // 2][:, (c % 2) * 512:(c % 2 + 1) * 512]
                        nc.tensor.matmul(
                            s_ps[:, csl],
                            lhsT=vt[j][:, qsl],
                            rhs=rhs,
                            start=(j == 0),
                            stop=(j == 1) and not add_bias,
                        )
                if add_bias:
                    # S += btab[qt] via identity matmul (PSUM accumulate)
                    bt = w_tiles[qt]
                    for c in range(4):
                        csl = slice(c * 512, (c + 1) * 512)
                        nc.tensor.matmul(
                            s_ps[:, csl], lhsT=idb, rhs=bt[:, csl],
                            start=False, stop=True,
                        )
                return s_ps

            pending_norms = []

            def emit_norm(b, qt, pw, rinv, eng):
                qsl = slice(qt * 128, (qt + 1) * 128)
                out_t = op_pool.tile([128, N], BF16, tag="o", name=f"o{b}{qt}")
                if eng == "act":
                    nc.scalar.mul(out=out_t, in_=pw, mul=rinv)
                else:
                    nc.vector.tensor_scalar(
                        out=out_t, in0=pw, scalar1=rinv, scalar2=None,
                        op0=mybir.AluOpType.mult,
                    )
                nc.gpsimd.dma_start(out=o_d[b, qsl, :], in_=out_t)

            def flush_pending():
                while pending_norms:
                    emit_norm(*pending_norms.pop(0), "act")

            def emit_qtile_rest(b, qt, s_ps, norm_eng="dve"):
                add_bias = _is_add(b, qt)
                p1 = pp.tile([128, N], FP16, tag="p", name=f"p{b}{qt}")
                rs = rsp.tile([128, 1], F32, tag="rs", name=f"rs{b}{qt}")
                if add_bias:
                    # bias already in S: exp accumulates the rowsum itself
                    nc.scalar.activation(
                        out=p1, in_=s_ps,
                        func=mybir.ActivationFunctionType.Exp,
                        accum_out=rs,
                    )
                    flush_pending()
                    pw = p1
                else:
                    nc.scalar.activation(
                        out=p1, in_=s_ps,
                        func=mybir.ActivationFunctionType.Exp,
                    )
                    # ACT-normalizes of older tiles go right behind this exp
                    flush_pending()
                    # pw = (p1*1+0)*w[qt], rs = rowsum (fused custom DVE op)
                    pw = pwp.tile([128, N], FP16, tag="pw", name=f"pw{b}{qt}")
                    nc.vector.affine_mul_reduce(
                        out=pw, accum_out=rs, in0=p1, in1=w_tiles[qt],
                        scale=1.0, bias=0.0,
                    )
                rinv = rsp.tile([128, 1], F32, tag="rinv", name=f"ri{b}{qt}")
                nc.vector.reciprocal(out=rinv, in_=rs)
                if norm_eng == "act":
                    pending_norms.append((b, qt, pw, rinv))
                else:
                    emit_norm(b, qt, pw, rinv, "dve")

            def emit_qtile(b, qt):
                eng = "act" if qt in ACT_NORM_QT else "dve"
                emit_qtile_rest(b, qt, emit_qtile_s(b, qt), eng)

            # ---- pipeline: batch-major, VT(b+1) built just after (b, qt1)
            # (adjacent vp pair keeps the PSUM ring parity intact) ----
            vts[0] = emit_vt(0)          # DVE copies (idle at ramp)
            s00 = emit_qtile_s(0, 0)
            emit_w(4, "gps")
            emit_qtile_rest(0, 0, s00)
            emit_qtile(0, 1)
            vts[1] = emit_vt(1, ("act", "act"))
            emit_w(5, "gps")
            emit_qtile(0, 2)
            emit_hnt(2)
            emit_qtile(0, 3)
            emit_qtile(0, 4)
            emit_qtile(0, 5)
            emit_qtile(0, 6)
            emit_qtile(0, 7)

            for b in range(1, NB):
                for qt in range(NQT):
                    emit_qtile(b, qt)
                    if qt == 1 and b + 1 < NB:
                        vts[b + 1] = emit_vt(b + 1, ("act", "act"))
                    if qt == 3 and b + 2 < NB:
                        emit_hnt(b + 2)
            flush_pending()
            # keep the PE p-state at full duty through the drain window
            # (the k=4 half-duty window otherwise starts the moment the
            # last real matmul retires, stretching the teardown drains)
            wtail = ps_s.tile([128, N], F32, tag="s", name="warmtail")
            for _ in range(12):
                nc.tensor.matmul(
                    wtail[:, :128], lhsT=wrm, rhs=wrm, start=True, stop=True,
                    skip_group_check=True,
                )
    nc.compile()
    return nc


def _get_nc():
    if "nc" not in _CACHE:
        _CACHE["nc"] = _build()
    return _CACHE["nc"]


def _swap_halves(x):
    # swap the two key-column halves (involution)
    return np.concatenate([x[..., NQ:], x[..., :NQ]], axis=-1)


def make_in_maps(Hn, A_stat, M_mask, Wq, Wk):
    Hn = np.ascontiguousarray(np.asarray(Hn, dtype=np.float32))
    A_stat = np.asarray(A_stat, dtype=np.float32)
    M_mask = np.asarray(M_mask)
    Wq = np.asarray(Wq, dtype=np.float32)
    Wk = np.asarray(Wk, dtype=np.float32)
    assert Hn.shape == (B_FULL, N, D)

    # multiplicative bias: w = mask * a/(1-a), a = clip(A, eps, 1-eps)
    # additive form (ADD_QT slots): btab = ln(w) = logit(a); masked -> -69
    a = np.clip(A_stat, EPS, 1.0 - EPS)
    mask = M_mask != 0
    w_full = np.where(mask, a / (1.0 - a), 0.0).astype(np.float16)
    btab_full = np.where(
        mask, np.log(a) - np.log1p(-a), np.float32(-69.0)
    ).astype(np.float16)

    # G = (Wq^T Wk)/sqrt(D), packed [p, i, e] = G[i*128+p, e]
    G = (Wq.T @ Wk) * SCALE
    g_packed = np.ascontiguousarray(
        G.reshape(2, 128, D).transpose(1, 0, 2).astype(np.float16)
    )

    # [16, 256, 2048] transposed-node layout, fp16
    hnt_full = np.ascontiguousarray(Hn.astype(np.float16).transpose(0, 2, 1))

    in_maps = []
    for c in range(8):
        bg, qg = c // NQG, c % NQG
        bsl = slice(bg * NB, (bg + 1) * NB)
        qsl = slice(qg * NQ, (qg + 1) * NQ)
        hnt_c = hnt_full[bsl]
        w_c = w_full[qsl].reshape(NQT, 128, N).copy()
        b_c = btab_full[qsl].reshape(NQT, 128, N)
        for t in ADD_QT:
            w_c[t] = b_c[t]
        if qg == 1:
            hnt_c = _swap_halves(hnt_c)
            w_c = _swap_halves(w_c)
        in_maps.append({
            "hnt": np.ascontiguousarray(hnt_c),
            "w": np.ascontiguousarray(w_c),
            "g": g_packed,
            "idb": np.eye(128, dtype=np.float16),
        })
    return in_maps


def assemble(results):
    out = np.empty((B_FULL, N, N), dtype=np.float32)
    for c in range(8):
        bg, qg = c // NQG, c % NQG
        o = np.asarray(results[c]["o"])
        if qg == 1:
            o = _swap_halves(o)
        out[bg * NB:(bg + 1) * NB, qg * NQ:(qg + 1) * NQ, :] = (
            o.astype(np.float32)
        )
    return out


def kernel(Hn, A_stat, M_mask, Wq, Wk):
    in_maps = make_in_maps(Hn, A_stat, M_mask, Wq, Wk)
    nc = _get_nc()
    res = bass_utils.run_bass_kernel_spmd(nc, in_maps, core_ids=list(range(8)))
    return assemble(res.results)


if __name__ == "__main__":
    rng = np.random.default_rng(0)
    bound = 1.0 / math.sqrt(D)  # nn.Linear default init, as in the problem
    inputs = {
        "Hn": rng.standard_normal((B_FULL, N, D), dtype=np.float32),
        "A_stat": rng.random((N, N), dtype=np.float32),
        "M_mask": rng.integers(0, 2, size=(N, N), dtype=np.int32),
        "Wq": rng.uniform(-bound, bound, (D, D)).astype(np.float32),
        "Wk": rng.uniform(-bound, bound, (D, D)).astype(np.float32),
    }
    out = kernel(**inputs)
    print(out.shape, out.dtype, out.sum())


# revision 51
# speedup vs baseline: 1.0150x; 1.0150x over previous
"""
Trainium2 Bass kernel for DynamicGraphAttention
(softmax(Hn Wq^T (Hn Wk^T)^T / sqrt(D) + eta*logit(clip(A)) masked)).

Shapes (hardcoded):
  Hn     [16, 2048, 256] f32
  A_stat [2048, 2048]    f32
  M_mask [2048, 2048]    int32
  Wq, Wk [256, 256]      f32
  out    [16, 2048, 2048] f32

Sharding across 8 NeuronCores: 4 batch-groups x 2 seq(query)-groups.
Core c handles batches of group bg = c // 2 (4 batches) and query rows
[qg*1024:(qg+1)*1024] (qg = c % 2). The program is identical on all
cores (SPMD): for qg=1 cores the host swaps the two key-column halves
of hnt and w so the core's own query block is always columns [0:1024],
and the assemble step swaps the output's key axis back.

Hybrid bias application, chosen per qt slot to balance PE/ACT/DVE:
  mult slots: softmax(S+b) == exp(S)*w / sum(exp(S)*w) with
      w = mask * a/(1-a)  (fp16, host-computed; masked -> exactly 0).
      DVE AFFINE_MUL_REDUCE fuses pw = exp(S)*w with the rowsum.
  ADD_QT slots: btab = ln(w) is added into PSUM via an identity
      matmul on the PE (which has spare throughput there), and the
      ACT exp accumulates the rowsum for free.

G = (Wq^T Wk)/sqrt(D) is folded host-side (weight-only preprocessing,
fp16 [128,2,256]); Hn ships pre-transposed fp16 and the query block is
a column-slice view of it (no separate hqt load).

Device algorithm (per core):
  VT   = G^T HqT  per batch, fp16          [256,1024]        (PE)
  S    = VT.T @ HnT  (fp16 matmuls) PSUM f32 (+btab on ADD)  (PE)
  e    = exp(S) -> SBUF fp16 (+rowsum accum on ADD slots)    (ACT)
  pw   = e * w[qt], rs = rowsum(pw)   (AFFINE_MUL_REDUCE)    (DVE)
  out  = pw * (1/rs)  bf16 -> DRAM                           (DVE)

Emission order = per-engine execution order; VT builds (PSUM vp pair
adjacent to keep ring parity; casts on ACT so the DVE FIFO can't delay
them) and the ring-split input DMAs are interleaved into the sweep.
Output is bf16 on device (fp16 lacks exponent range for the smallest
softmax probabilities), upcast to f32 on host.
"""

import math

import numpy as np

import concourse.bass as bass
import concourse.bacc as bacc
import concourse.tile as tile
from concourse import mybir
from concourse import bass_utils

F32 = mybir.dt.float32
FP16 = mybir.dt.float16
BF16 = mybir.dt.bfloat16

B_FULL = 16
N = 2048
D = 256
NBG = 4   # batch groups
NQG = 2   # seq (query-row) groups
NB = B_FULL // NBG        # batches per core = 4
NQ = N // NQG             # query rows per core = 1024
NQT = NQ // 128           # q tiles per core = 8
EPS = 1e-3
SCALE = 1.0 / math.sqrt(float(D))  # 1/16

# qt slots using the additive ln(w) bias via PE identity-matmul (the rest
# multiply by w on the DVE); qt slots whose normalize runs on ACT.
# Chosen to balance PE / ACT / DVE busy time.
ADD_QT = (1, 4, 7)
ACT_NORM_QT = ()


def _is_add(b, qt):
    return qt in ADD_QT

_CACHE = {}


def _patch_act_tables():
    # Prefer the activation-table set that holds Exp (+Copy) so the
    # scalar engine never reloads tables mid-kernel.
    from concourse import hw_specs as _hw
    if getattr(_hw, "_combined_first", False):
        return
    _orig = _hw.get_activation_tables

    def _patched(module_arch):
        tabs = _orig(module_arch)
        pref = "natural_log_exp_and_others"
        if pref in tabs:
            both = {mybir.ActivationFunctionType.Ln,
                    mybir.ActivationFunctionType.Exp,
                    mybir.ActivationFunctionType.Copy}
            tabs = {
                k: (v if k == pref else (v - both))
                for k, v in tabs.items()
            }
        return tabs

    _hw.get_activation_tables = _patched
    import concourse.bacc as _bacc_mod
    _bacc_mod.get_activation_tables = _patched
    _hw._combined_first = True


def _build():
    _patch_act_tables()
    nc = bacc.Bacc("TRN2", debug=False, enable_asserts=False)

    hnt_d = nc.dram_tensor("hnt", [NB, D, N], FP16, kind="ExternalInput").ap()
    # per-qt-slot bias sheet: ln(w) for ADD_QT slots, w for the others
    w_d = nc.dram_tensor("w", [NQT, 128, N], FP16, kind="ExternalInput").ap()
    g_d = nc.dram_tensor("g", [128, 2, D], FP16, kind="ExternalInput").ap()
    idb_d = nc.dram_tensor("idb", [128, 128], FP16, kind="ExternalInput").ap()
    o_d = nc.dram_tensor("o", [NB, NQ, N], BF16, kind="ExternalOutput").ap()

    with tile.TileContext(nc) as tc:
        with (
            tc.tile_pool(name="consts", bufs=1) as consts,
            tc.tile_pool(name="wp", bufs=NQT) as wp,
            tc.tile_pool(name="hntp", bufs=12) as hntp,
            tc.tile_pool(name="vtp", bufs=4) as vtp,
            tc.tile_pool(name="pp", bufs=3) as pp,
            tc.tile_pool(name="pwp", bufs=3) as pwp,
            tc.tile_pool(name="op", bufs=3) as op_pool,
            tc.tile_pool(name="rsp", bufs=8) as rsp,
            tc.tile_pool(name="ps_s", bufs=2, space="PSUM") as ps_s,
        ):
            # ---- PE p-state warmup, first thing in every engine stream:
            # the full-speed p-state window opens only after sustained PE
            # activity, so the warmup matmuls must not wait on a DVE memset.
            # They read wrm before its memset on purpose (weight values
            # are irrelevant; the PSUM slot is reset by later start=True).
            wrm = consts.tile([128, 128], FP16, tag="wrm")
            wps = ps_s.tile([128, N], F32, tag="s", name="warmps")
            for _ in range(18):
                nc.tensor.matmul(
                    wps[:, :128], lhsT=wrm, rhs=wrm, start=True, stop=True,
                    skip_group_check=True,
                )

            # ---- tiny const + ACT table preload (Exp) ----
            tinyc = consts.tile([128, 1], F32, tag="tiny")
            nc.vector.memset(tinyc, 1e-30)
            nc.vector.memset(wrm, 0.5)
            warm = consts.tile([128, 1], F32, tag="warm")
            nc.scalar.activation(
                out=warm, in_=tinyc,
                func=mybir.ActivationFunctionType.Exp, bias=0.0, scale=1.0,
            )

            # ---- constants ----
            g_sb = consts.tile([128, 2, D], FP16, tag="g")
            nc.sync.dma_start(out=g_sb, in_=g_d)
            idb = consts.tile([128, 128], FP16, tag="idb")
            nc.sync.dma_start(out=idb, in_=idb_d)

            # DMA rings: sync + act are HWDGE, gps is SWDGE (also carries
            # the output stores). Ramp loads are split across all three.
            rings = {"sync": nc.sync, "act": nc.scalar, "gps": nc.gpsimd}

            # w tiles
            w_tiles = {}

            def emit_w(t, ring="gps"):
                w_t = wp.tile([128, N], FP16, tag="w", name=f"w{t}")
                rings[ring].dma_start(out=w_t, in_=w_d[t])
                w_tiles[t] = w_t

            # hnt[b] as 4 subtiles [i][h]: i = d'-row block, h = col half.
            # Half 0 is always the core's own query block (host-permuted),
            # loaded first so VT can start early.
            hnts = {}

            def emit_hnt(b, half=None, ring="sync"):
                halves = [0, 1] if half is None else [half]
                if b not in hnts:
                    hnts[b] = [[None, None], [None, None]]
                for h in halves:
                    for i in range(2):
                        t = hntp.tile([128, NQ], FP16, tag="hnt",
                                      name=f"hnt{b}_{i}{h}")
                        rings[ring].dma_start(
                            out=t,
                            in_=hnt_d[b, i * 128:(i + 1) * 128,
                                      h * NQ:(h + 1) * NQ],
                        )
                        hnts[b][i][h] = t
                return hnts[b]

            # preamble loads spread over the three rings so batch 0+1 and
            # the first bias sheets land as fast as possible; b0h0's two
            # subtiles ride different rings so they land in parallel
            t00 = hntp.tile([128, NQ], FP16, tag="hnt", name="hnt0_00")
            nc.gpsimd.dma_start(out=t00, in_=hnt_d[0, 0:128, 0:NQ])
            t01 = hntp.tile([128, NQ], FP16, tag="hnt", name="hnt0_10")
            nc.scalar.dma_start(out=t01, in_=hnt_d[0, 128:256, 0:NQ])
            hnts[0] = [[t00, None], [t01, None]]
            emit_hnt(0, half=1, ring="act")
            emit_w(1, "gps")
            emit_w(0, "gps")

            emit_hnt(1, half=0, ring="sync")
            emit_hnt(1, half=1, ring="sync")
            emit_w(2, "gps")
            emit_w(3, "gps")
            emit_w(6, "act")
            emit_w(7, "act")

            # ---- VT builder: VT[d',q] = sum_d G[d,d'] HqT[d,q] ----
            # vp PSUM written in 512-col chunks (bank-sized matmuls);
            # copy engine selectable for load balancing.
            def emit_vt(b, engs=("dve", "dve")):
                vt = []
                hq = [hnts[b][i][0] for i in range(2)]
                for j in range(2):
                    vt_j = vtp.tile([128, NQ], FP16, tag="vt", name=f"vt{b}_{j}")
                    vp = ps_s.tile(
                        [128, N], F32, tag="s", name=f"vp{b}{j}"
                    )[:, :NQ]
                    for i in range(2):
                        for c in range(2):
                            csl = slice(c * 512, (c + 1) * 512)
                            nc.tensor.matmul(
                                vp[:, csl],
                                lhsT=g_sb[:, i, j * 128:(j + 1) * 128],
                                rhs=hq[i][:, csl],
                                start=(i == 0),
                                stop=(i == 1),
                            )
                    if engs[j] == "act":
                        nc.scalar.copy(out=vt_j, in_=vp)
                    else:
                        nc.vector.tensor_copy(out=vt_j, in_=vp)
                    vt.append(vt_j)
                return vt

            vts = {}

            def emit_qtile_s(b, qt):
                qsl = slice(qt * 128, (qt + 1) * 128)
                vt, hnt = vts[b], hnts[b]
                add_bias = _is_add(b, qt)
                s_ps = ps_s.tile([128, N], F32, tag="s", name=f"s{b}{qt}")
                for j in range(2):
                    for c in range(4):
                        csl = slice(c * 512, (c + 1) * 512)
                        rhs = hnt[j][c // 2][:, (c % 2) * 512:(c % 2 + 1) * 512]
                        nc.tensor.matmul(
                            s_ps[:, csl],
                            lhsT=vt[j][:, qsl],
                            rhs=rhs,
                            start=(j == 0),
                            stop=(j == 1) and not add_bias,
                        )
                if add_bias:
                    # S += btab[qt] via identity matmul (PSUM accumulate)
                    bt = w_tiles[qt]
                    for c in range(4):
                        csl = slice(c * 512, (c + 1) * 512)
                        nc.tensor.matmul(
                            s_ps[:, csl], lhsT=idb, rhs=bt[:, csl],
                            start=False, stop=True,
                        )
                return s_ps

            pending_norms = []

            def emit_norm(b, qt, pw, rinv, eng):
                qsl = slice(qt * 128, (qt + 1) * 128)
                out_t = op_pool.tile([128, N], BF16, tag="o", name=f"o{b}{qt}")
                if eng == "act":
                    nc.scalar.mul(out=out_t, in_=pw, mul=rinv)
                else:
                    nc.vector.tensor_scalar(
                        out=out_t, in0=pw, scalar1=rinv, scalar2=None,
                        op0=mybir.AluOpType.mult,
                    )
                nc.gpsimd.dma_start(out=o_d[b, qsl, :], in_=out_t)

            def flush_pending():
                while pending_norms:
                    emit_norm(*pending_norms.pop(0), "act")

            def emit_qtile_rest(b, qt, s_ps, norm_eng="dve"):
                add_bias = _is_add(b, qt)
                p1 = pp.tile([128, N], FP16, tag="p", name=f"p{b}{qt}")
                rs = rsp.tile([128, 1], F32, tag="rs", name=f"rs{b}{qt}")
                if add_bias:
                    # bias already in S: exp accumulates the rowsum itself
                    nc.scalar.activation(
                        out=p1, in_=s_ps,
                        func=mybir.ActivationFunctionType.Exp,
                        accum_out=rs,
                    )
                    flush_pending()
                    pw = p1
                else:
                    nc.scalar.activation(
                        out=p1, in_=s_ps,
                        func=mybir.ActivationFunctionType.Exp,
                    )
                    # ACT-normalizes of older tiles go right behind this exp
                    flush_pending()
                    # pw = (p1*1+0)*w[qt], rs = rowsum (fused custom DVE op)
                    pw = pwp.tile([128, N], FP16, tag="pw", name=f"pw{b}{qt}")
                    nc.vector.affine_mul_reduce(
                        out=pw, accum_out=rs, in0=p1, in1=w_tiles[qt],
                        scale=1.0, bias=0.0,
                    )
                rinv = rsp.tile([128, 1], F32, tag="rinv", name=f"ri{b}{qt}")
                nc.vector.reciprocal(out=rinv, in_=rs)
                if norm_eng == "act":
                    pending_norms.append((b, qt, pw, rinv))
                else:
                    emit_norm(b, qt, pw, rinv, "dve")

            def emit_qtile(b, qt):
                eng = "act" if qt in ACT_NORM_QT else "dve"
                emit_qtile_rest(b, qt, emit_qtile_s(b, qt), eng)

            # ---- pipeline: batch-major, VT(b+1) built just after (b, qt1)
            # (adjacent vp pair keeps the PSUM ring parity intact) ----
            vts[0] = emit_vt(0)          # DVE copies (idle at ramp)
            s00 = emit_qtile_s(0, 0)
            emit_w(4, "gps")
            emit_qtile_rest(0, 0, s00)
            emit_qtile(0, 1)
            vts[1] = emit_vt(1, ("act", "act"))
            emit_w(5, "gps")
            emit_qtile(0, 2)
            emit_hnt(2)
            emit_qtile(0, 3)
            emit_qtile(0, 4)
            emit_qtile(0, 5)
            emit_qtile(0, 6)
            emit_qtile(0, 7)

            for b in range(1, NB):
                for qt in range(NQT):
                    emit_qtile(b, qt)
                    if qt == 1 and b + 1 < NB:
                        vts[b + 1] = emit_vt(b + 1, ("act", "act"))
                    if qt == 3 and b + 2 < NB:
                        emit_hnt(b + 2)
            flush_pending()
    nc.compile()
    return nc


def _get_nc():
    if "nc" not in _CACHE:
        _CACHE["nc"] = _build()
    return _CACHE["nc"]


def _swap_halves(x):
    # swap the two key-column halves (involution)
    return np.concatenate([x[..., NQ:], x[..., :NQ]], axis=-1)


def make_in_maps(Hn, A_stat, M_mask, Wq, Wk):
    Hn = np.ascontiguousarray(np.asarray(Hn, dtype=np.float32))
    A_stat = np.asarray(A_stat, dtype=np.float32)
    M_mask = np.asarray(M_mask)
    Wq = np.asarray(Wq, dtype=np.float32)
    Wk = np.asarray(Wk, dtype=np.float32)
    assert Hn.shape == (B_FULL, N, D)

    # multiplicative bias: w = mask * a/(1-a), a = clip(A, eps, 1-eps)
    # additive form (ADD_QT slots): btab = ln(w) = logit(a); masked -> -69
    a = np.clip(A_stat, EPS, 1.0 - EPS)
    mask = M_mask != 0
    w_full = np.where(mask, a / (1.0 - a), 0.0).astype(np.float16)
    btab_full = np.where(
        mask, np.log(a) - np.log1p(-a), np.float32(-69.0)
    ).astype(np.float16)

    # G = (Wq^T Wk)/sqrt(D), packed [p, i, e] = G[i*128+p, e]
    G = (Wq.T @ Wk) * SCALE
    g_packed = np.ascontiguousarray(
        G.reshape(2, 128, D).transpose(1, 0, 2).astype(np.float16)
    )

    # [16, 256, 2048] transposed-node layout, fp16
    hnt_full = np.ascontiguousarray(Hn.astype(np.float16).transpose(0, 2, 1))

    in_maps = []
    for c in range(8):
        bg, qg = c // NQG, c % NQG
        bsl = slice(bg * NB, (bg + 1) * NB)
        qsl = slice(qg * NQ, (qg + 1) * NQ)
        hnt_c = hnt_full[bsl]
        w_c = w_full[qsl].reshape(NQT, 128, N).copy()
        b_c = btab_full[qsl].reshape(NQT, 128, N)
        for t in ADD_QT:
            w_c[t] = b_c[t]
        if qg == 1:
            hnt_c = _swap_halves(hnt_c)
            w_c = _swap_halves(w_c)
        in_maps.append({
            "hnt": np.ascontiguousarray(hnt_c),
            "w": np.ascontiguousarray(w_c),
            "g": g_packed,
            "idb": np.eye(128, dtype=np.float16),
        })
    return in_maps


def assemble(results):
    out = np.empty((B_FULL, N, N), dtype=np.float32)
    for c in range(8):
        bg, qg = c // NQG, c % NQG
        o = np.asarray(results[c]["o"])
        if qg == 1:
            o = _swap_halves(o)
        out[bg * NB:(bg + 1) * NB, qg * NQ:(qg + 1) * NQ, :] = (
            o.astype(np.float32)
        )
    return out


def kernel(Hn, A_stat, M_mask, Wq, Wk):
    in_maps = make_in_maps(Hn, A_stat, M_mask, Wq, Wk)
    nc = _get_nc()
    res = bass_utils.run_bass_kernel_spmd(nc, in_maps, core_ids=list(range(8)))
    return assemble(res.results)


if __name__ == "__main__":
    rng = np.random.default_rng(0)
    bound = 1.0 / math.sqrt(D)  # nn.Linear default init, as in the problem
    inputs = {
        "Hn": rng.standard_normal((B_FULL, N, D), dtype=np.float32),
        "A_stat": rng.random((N, N), dtype=np.float32),
        "M_mask": rng.integers(0, 2, size=(N, N), dtype=np.int32),
        "Wq": rng.uniform(-bound, bound, (D, D)).astype(np.float32),
        "Wk": rng.uniform(-bound, bound, (D, D)).astype(np.float32),
    }
    out = kernel(**inputs)
    print(out.shape, out.dtype, out.sum())
